# revision 1
# baseline (speedup 1.0000x reference)
"""Trainium2 Bass kernel for nn_BandpassFilter (first-order Butterworth
band-pass: high-pass(low_cutoff) + low-pass(high_cutoff), summed).

Math
----
The reference runs two coupled first-order IIR filters over T=262144 time
steps per waveform:  y[n] = b0*x[n] + b1*x[n-1] - a1*y[n-1]  (per filter,
zero initial state), output = y_hp + y_lp.

The combined impulse response is h[0] = bh0 + bl0 and, for d >= 1,
h[d] = ch*Ah^(d-1) + cl*Al^(d-1)  with  Af = -af1, cf = bf1 - af1*bf0.
|Ah| ~ 0.972, |Al| ~ 0.867 for the given cutoffs, so h decays below the
fp32 noise floor by d ~ 500.  The IIR therefore equals (to fp32 accuracy)
a causal FIR convolution with ~512 taps — which maps onto the TensorEngine
with NO sequential scan via a polyphase decomposition:

  t = 128*j + p   (p = phase = partition, j = column)
  y[q, j] = sum_{m=0..3} sum_p  Wm[q, p] * x[p, j - m]
  Wm[q, p] = h[q - p + 128*m]          (taps d in [0, 511])

i.e. 4 accumulating [128k x 128m x 512n] matmuls per 512-column tile,
contraction over the 128 phases.  Shifted columns (j-m) are plain AP
offsets into the phase-major SBUF buffer (zero-padded on the left, which
reproduces the zero initial conditions exactly).

Layout: phase-major requires a 128x128 transpose on the way in and out;
both are done on the TensorEngine (transpose mode), with PSUM->SBUF
copies split between VectorE and ScalarE.  All matmuls/transposes run as
float32r (full-rate fp32 mode of the PE).

Sharding: batch dim (64 waveforms) split 8 ways across the 8 NeuronCores;
the filter is per-waveform so there is no cross-core communication.
"""

import numpy as np

SAMPLE_RATE = 44100.0
B_FULL = 64
T = 262144
NCORES = 8
RPC = B_FULL // NCORES  # rows (waveforms) per core
P = 128                 # phases == partitions
J = T // P              # 2048 phase-major columns per row
JC = J // P             # 16 column-chunks of 128 (transpose granularity)
NTILE = J // 512        # 4 conv tiles of 512 columns
NLAGS = 4               # m = 0..3  ->  taps d in [0, 511]
PAD = 4                 # left zero-padding columns (>= NLAGS-1; 4 for ISA-friendly memset width)


def _coeffs(low_cutoff, high_cutoff):
    """butter(1, wn) coefficients, mirroring the fp32 arithmetic of the
    reference (bilinear transform)."""
    f32 = np.float32
    nyq = f32(SAMPLE_RATE / 2.0)
    low = np.clip(f32(low_cutoff), f32(0.0), nyq)
    high = np.clip(f32(high_cutoff), low, nyq)

    def butter1(wn, btype):
        t = np.tan(f32(np.pi) * wn / f32(2.0))
        a1 = (t - f32(1.0)) / (t + f32(1.0))
        if btype == "low":
            b0 = t / (f32(1.0) + t)
            b1 = b0
        else:
            b0 = f32(1.0) / (f32(1.0) + t)
            b1 = -b0
        return b0, b1, a1

    bh0, bh1, ah1 = butter1(low / nyq, "high")
    bl0, bl1, al1 = butter1(high / nyq, "low")
    return (bh0, bh1, ah1), (bl0, bl1, al1)


def _impulse_response(low_cutoff, high_cutoff, n):
    (bh0, bh1, ah1), (bl0, bl1, al1) = _coeffs(low_cutoff, high_cutoff)
    # exact powers in float64 of the fp32 coefficients
    Ah, Al = -np.float64(ah1), -np.float64(al1)
    ch = np.float64(bh1) - np.float64(ah1) * np.float64(bh0)
    cl = np.float64(bl1) - np.float64(al1) * np.float64(bl0)
    d = np.arange(1, n)
    h = np.empty(n, np.float64)
    h[0] = np.float64(bh0) + np.float64(bl0)
    h[1:] = ch * Ah ** (d - 1) + cl * Al ** (d - 1)
    return h


def _weights(low_cutoff, high_cutoff):
    """W tensor, already transposed for the matmul's lhsT operand:
    w[m, p, q] = h[q - p + 128*m]  (zero where the index is negative)."""
    h = _impulse_response(low_cutoff, high_cutoff, NLAGS * P)
    q = np.arange(P)[None, :]
    p = np.arange(P)[:, None]
    w = np.zeros((NLAGS, P, P), np.float64)
    for m in range(NLAGS):
        d = q - p + P * m
        valid = d >= 0
        w[m][valid] = h[d[valid]]
    return w.astype(np.float32)


_BUILD_CACHE = {}


def _legalize_waits(nc, mybir):
    """This walrus build accepts at most ONE sync-wait per instruction.
    Tile emits several on some instructions (DMA lane FIFO + slot release
    etc.); split the extras into standalone single-wait EventSemaphore
    instructions on the same engine queue, which preserves ordering."""
    n = 0
    for fn in nc.m.functions:
        for blk in fn.blocks:
            new = []
            for inst in blk.instructions:
                si = getattr(inst, "sync_info", None)
                if si is not None and si.on_wait and len(si.on_wait) > 1:
                    waits = list(si.on_wait)
                    for w in waits[:-1]:
                        n += 1
                        new.append(mybir.InstEventSemaphore(
                            name=f"wsplit-{n}-{inst.name}",
                            engine=inst.engine,
                            ins=[], outs=[],
                            sync_info=mybir.SyncInfo(on_wait=[w],
                                                     on_update=[]),
                        ))
                    inst.sync_info = mybir.SyncInfo(
                        on_wait=[waits[-1]],
                        on_update=list(si.on_update or []))
                new.append(inst)
            blk.instructions = new
    return n


def build_nc(reps=1, legalize=True, loop_n=1):
    """Build the per-core Bass program (identical on all 8 cores).
    loop_n > 1 wraps the body in a hardware For_i loop (timing builds)."""
    key = (reps, legalize, loop_n)
    if key in _BUILD_CACHE:
        return _BUILD_CACHE[key]

    import concourse.bass as bass
    import concourse.mybir as mybir
    from concourse import tile
    from contextlib import ExitStack

    f32 = mybir.dt.float32
    f32r = mybir.dt.float32r

    nc = bass.Bass()
    x_in = nc.declare_dram_parameter("x", [RPC, T], f32, isOutput=False)
    w_in = nc.declare_dram_parameter("w", [NLAGS, P, P], f32, isOutput=False)
    id_in = nc.declare_dram_parameter("ident", [P, P], f32, isOutput=False)
    zp_in = nc.declare_dram_parameter("zpad", [P, PAD], f32, isOutput=False)
    y_out = nc.declare_dram_parameter("y", [RPC, T], f32, isOutput=True)

    with tile.TileContext(nc) as tc, ExitStack() as ctx:
        const = ctx.enter_context(tc.tile_pool(name="const", bufs=1))
        xn_pool = ctx.enter_context(tc.tile_pool(name="xn", bufs=2))
        xt_pool = ctx.enter_context(tc.tile_pool(name="xt", bufs=2))
        xl_pool = ctx.enter_context(tc.tile_pool(name="xl", bufs=2))
        ys_pool = ctx.enter_context(tc.tile_pool(name="ys", bufs=2))
        yt_pool = ctx.enter_context(tc.tile_pool(name="yt", bufs=2))
        psi_pool = ctx.enter_context(
            tc.tile_pool(name="psi", bufs=2, space="PSUM"))
        psy_pool = ctx.enter_context(
            tc.tile_pool(name="psy", bufs=4, space="PSUM"))
        pso_pool = ctx.enter_context(
            tc.tile_pool(name="pso", bufs=2, space="PSUM"))

        # Full-precision weights come in as fp32; the fp32r hi/lo split is
        # done ON DEVICE so the split matches the hardware's own f32r
        # rounding exactly:  w_h = f32r(w),  w0_l = f32r(w0 - w0_h).
        w_f = const.tile([P, NLAGS * P], f32)   # [p, (m q)] full fp32
        w_h = const.tile([P, NLAGS * P], f32r)  # f32r-rounded weights
        w0l = const.tile([P, P], f32r)          # m=0 residual
        w0d = const.tile([P, P], f32)           # fp32 scratch for residual
        id_f = const.tile([P, P], f32)          # identity for transposes
        nc.scalar.dma_start(out=id_f[:], in_=id_in[:])
        nc.scalar.dma_start(
            out=w_f[:].rearrange("p (m q) -> p m q", q=P),
            in_=w_in.rearrange("m p q -> p m q"),
        )
        zc = const.tile([P, PAD], f32r)
        nc.gpsimd.dma_start(out=zc[:], in_=zp_in[:])
        nc.vector.tensor_copy(w_h[:], w_f[:])              # fp32 -> f32r round
        nc.vector.tensor_sub(w0d[:], w_f[:, 0:P], w_h[:, 0:P])
        nc.vector.tensor_copy(w0l[:], w0d[:])              # residual -> f32r

        # warm-up: absorb each constant-DMA semaphore tick into the PE
        # vector clock with single-wait instructions.  Every fp32r matmul
        # self-loads its weights, so the lowered instruction has exactly
        # ONE sync-wait slot; bf16 dummy LDWEIGHTS ops ("pe_dep") absorb
        # cross-engine ticks so real matmuls only carry their PSUM-bank
        # WAW wait.
        bf16 = mybir.dt.bfloat16

        def pe_dep(ap):
            nc.tensor.ldweights(ap.bitcast(bf16))

        warm_f = pso_pool.tile([P, 512], f32, tag="pso")
        nc.tensor.transpose(warm_f[:, 0:P], id_f[:], id_f[:])
        warm_y = psy_pool.tile([P, 512], f32, tag="psy")
        nc.tensor.matmul(warm_y[:, 0:P], w_h[:, 0:P], w_h[:, 0:P],
                         start=True, stop=True)
        pe_dep(w0l[:, 0:1])

        def copy_dve(out, in_):
            return nc.vector.tensor_copy(out, in_)

        def copy_act(out, in_):
            return nc.scalar.copy(out, in_)

        prev_yt = None
        if loop_n > 1:
            ctx.enter_context(tc.For_i(0, loop_n, 1, staggered_reset=True))
        for rep in range(reps):
            for r in range(RPC):
                # ---- load (natural layout: partition = j%128, free = (c,p))
                xn = xn_pool.tile([P, J], f32, tag="xn")
                xn3 = xn[:].rearrange("j (c p) -> j c p", p=P)
                xr3 = x_in[r].rearrange("(c j p) -> j c p", j=P, p=P)
                if r == 0:
                    # first row: quarter DMAs so the very first transposes
                    # start a quarter-transfer earlier
                    qc = JC // 4
                    for g in range(4):
                        nc.sync.dma_start(out=xn3[:, g * qc:(g + 1) * qc],
                                          in_=xr3[:, g * qc:(g + 1) * qc])
                else:
                    nc.sync.dma_start(out=xn3[:, 0:JC // 2],
                                      in_=xr3[:, 0:JC // 2])
                    nc.sync.dma_start(out=xn3[:, JC // 2:JC],
                                      in_=xr3[:, JC // 2:JC])
                xt = xt_pool.tile([P, PAD + J], f32r, tag="xt")
                xl = xl_pool.tile([P, PAD + J], f32r, tag="xl")
                nc.vector.tensor_copy(xt[:, 0:PAD], zc[:])
                nc.vector.tensor_copy(xl[:, 0:PAD], zc[:])
                ys = ys_pool.tile([P, J], f32, tag="ys")
                yt = yt_pool.tile([P, J], f32, tag="yt")

                pe_dep(xn[:, 0:1])  # absorb the x DMA tick

                def transpose_group(g):
                    psi = psi_pool.tile([P, 512], f32, tag="psi")
                    for k in range(4):
                        jc = g * 4 + k
                        nc.tensor.transpose(
                            psi[:, k * P:(k + 1) * P],
                            xn[:, jc * P:(jc + 1) * P],
                            id_f[:],
                        )
                    sl = slice(PAD + g * 512, PAD + (g + 1) * 512)
                    copy_dve(xt[:, sl], psi[:])          # fp32 -> f32r (hi)
                    nc.vector.tensor_sub(xl[:, sl], psi[:], xt[:, sl])

                def conv_tile(jt):
                    pe_dep(xl[:, PAD + jt * 512:PAD + jt * 512 + 1])
                    psy = psy_pool.tile([P, 512], f32, tag="psy")
                    b0 = PAD + jt * 512
                    nc.tensor.matmul(psy[:], w_h[:, 0:P],
                                     xt[:, b0:b0 + 512],
                                     start=True, stop=False)
                    nc.tensor.matmul(psy[:], w_h[:, 0:P],
                                     xl[:, b0:b0 + 512],
                                     start=False, stop=False)
                    nc.tensor.matmul(psy[:], w0l[:],
                                     xt[:, b0:b0 + 512],
                                     start=False, stop=False)
                    for m in range(1, NLAGS):
                        base = b0 - m
                        nc.tensor.matmul(
                            psy[:],
                            w_h[:, m * P:(m + 1) * P],
                            xt[:, base:base + 512],
                            start=False,
                            stop=(m == NLAGS - 1),
                        )
                    copy_dve(ys[:, jt * 512:(jt + 1) * 512], psy[:])

                def out_group(g):
                    pe_dep(ys[:, g * 512:g * 512 + 1])
                    if g >= 2:
                        # pso slot reuse within the row: absorb the
                        # out-copy (DVE) release tick too
                        pe_dep(yt[:, (g - 2) * 512:(g - 2) * 512 + 1])
                    elif prev_yt is not None:
                        # pso slot reuse across rows: absorb the previous
                        # row's out-copy g+2 release tick
                        pe_dep(prev_yt[:, (g + 2) * 512:(g + 2) * 512 + 1])
                    pso = pso_pool.tile([P, 512], f32, tag="pso")
                    for k in range(4):
                        jb = g * 4 + k
                        nc.tensor.transpose(
                            pso[:, k * P:(k + 1) * P],
                            ys[:, jb * P:(jb + 1) * P],
                            id_f[:],
                        )
                    copy_act(yt[:, g * 512:(g + 1) * 512], pso[:])

                transpose_group(0)
                transpose_group(1)
                conv_tile(0)
                transpose_group(2)
                conv_tile(1)
                transpose_group(3)
                conv_tile(2)
                conv_tile(3)
                yo3 = y_out[r].rearrange("(c j q) -> j c q", j=P, q=P)
                yt3 = yt[:].rearrange("j (c q) -> j c q", q=P)
                if r < RPC - 1:
                    out_group(0)
                    out_group(1)
                    nc.scalar.dma_start(out=yo3[:, 0:JC // 2],
                                        in_=yt3[:, 0:JC // 2])
                    out_group(2)
                    out_group(3)
                    nc.scalar.dma_start(out=yo3[:, JC // 2:JC],
                                        in_=yt3[:, JC // 2:JC])
                else:
                    # last row: quarter DMAs on the (idle) SP ring so the
                    # kernel tail ends ~a quarter-transfer after the final
                    # out-copy instead of a half-transfer behind ACT
                    qc = JC // 4
                    for g in range(4):
                        out_group(g)
                        nc.sync.dma_start(
                            out=yo3[:, g * qc:(g + 1) * qc],
                            in_=yt3[:, g * qc:(g + 1) * qc])
                prev_yt = yt

    if legalize:
        _legalize_waits(nc, mybir)
    _BUILD_CACHE[key] = nc
    return nc


def kernel(x, low_cutoff, high_cutoff):
    from concourse.bass_utils import run_bass_kernel_spmd

    x = np.asarray(x, dtype=np.float32)
    w = _weights(np.asarray(low_cutoff), np.asarray(high_cutoff))
    ident = np.eye(P, dtype=np.float32)

    nc = build_nc(reps=1)
    in_maps = [
        {"x": np.ascontiguousarray(x[c * RPC:(c + 1) * RPC]),
         "w": w, "ident": ident, "zpad": np.zeros((P, PAD), np.float32)}
        for c in range(NCORES)
    ]
    res = run_bass_kernel_spmd(nc, in_maps, list(range(NCORES)))
    return np.concatenate([res.results[c]["y"] for c in range(NCORES)], axis=0)

